# revision 9
# baseline (speedup 1.0000x reference)
"""Bilinear pooling kernel for Trainium2 (8 NeuronCores, data-parallel over batch).

reference:
    xp = x @ W.T          [B, 2048]
    yp = y @ W.T          [B, 2048]
    z[b] = flatten(outer(xp[b], yp[b]))    [B, 2048*2048]
    out = z / max(||z||_2, 1e-12)  (row-wise L2 normalize)

Key identity: ||outer(xp, yp)||_F = ||xp||_2 * ||yp||_2, so the normalizer is
computed from xp/yp directly. The 512MB output is written once, as float16
(rel err ~5e-4, well under the 2e-2 gate), halving HBM write traffic.

Per-core plan (4 samples each):
  1. DMA x,y shards; PE-transpose into xyT16 [128, 8, 8] fp16 (i on partitions).
  2. Stream W in 4x 2MB chunks; DVE-convert to fp16; XBAR dma_start_transpose
     128-row groups into wt [128, ob, k, 128] fp16 (no PE transposes for W);
     fp16 matmuls xp/yp -> xy_proj [8, 2048] f32; incremental PE transposes of
     proj columns into xpT2 [128, c2, jj, 8] (pair-packed: o = c2*256+2p+jj).
  3. Norms via tensor_tensor_reduce; s_b = 1/max(||xp_b||*||yp_b||, eps);
     scaled fp16 yp rows; DMA partition-broadcast to ypb[b] [128, 2048] fp16.
  4. Outer products: ot[p, jj, :] = ypb[b] * xpT2[:, c2, jj, b] (DVE/ACT fp16);
     1MB DMAs out with 8KB contiguous per partition line.
"""

import sys
from contextlib import ExitStack

import numpy as np

if "/opt/trn_rl_repo" not in sys.path:
    sys.path.insert(0, "/opt/trn_rl_repo")

B, D_IN, D_OUT = 32, 1024, 2048
NCORES = 8
BL = B // NCORES  # 4 samples per core
P = 128
KC = D_IN // P  # 8 contraction chunks
OB = D_OUT // P  # 16 o-blocks of 128 rows
NCH = 4  # W streamed in 4 chunks of 512 rows
C2 = 8  # output chunks per sample: o in [c2*256, (c2+1)*256), pair-packed
EPS = 1e-12

_cache = {}


def _build_nc(debug_stop=None):
    import concourse.bass as bass  # noqa: F401
    import concourse.mybir as mybir
    import concourse.tile as tile
    from concourse import bacc
    from concourse.masks import make_identity

    f32 = mybir.dt.float32
    f16 = mybir.dt.float16
    nc = bacc.Bacc()

    x_ext = nc.declare_dram_parameter("x", [BL, D_IN], f32, isOutput=False)
    y_ext = nc.declare_dram_parameter("y", [BL, D_IN], f32, isOutput=False)
    w_ext = nc.declare_dram_parameter("W", [D_OUT, D_IN], f32, isOutput=False)
    if debug_stop is None:
        out_ext = nc.declare_dram_parameter(
            "out", [BL, D_OUT * D_OUT], f16, isOutput=True
        )
        # out row b, flat index o*2048 + f with o = c2*256 + 2*p + jj
        out_r = out_ext[:].rearrange(
            "b (c2 p jj f) -> b c2 p jj f", c2=C2, p=P, jj=2, f=D_OUT
        )
    elif debug_stop == "proj":
        dbg_proj = nc.declare_dram_parameter(
            "dbg_proj", [2 * BL, D_OUT], f32, isOutput=True
        )
    elif debug_stop == "pre":
        dbg_xpt = nc.declare_dram_parameter(
            "dbg_xpt", [P, C2, 2, 2 * BL], f32, isOutput=True
        )
        dbg_ypb = nc.declare_dram_parameter(
            "dbg_ypb", [BL, P, D_OUT], f16, isOutput=True
        )

    # W row o = c*512 + g*128 + p
    w_r = w_ext[:].rearrange("(c g p) i -> c p g i", c=NCH, g=4, p=P)

    with tile.TileContext(nc) as tc:
        with (
            tc.tile_pool(name="const", bufs=1) as const_pool,
            tc.tile_pool(name="persist", bufs=1) as persist,
            tc.tile_pool(name="small_psum", bufs=2, space="PSUM") as small_psum,
        ):
            ident8 = const_pool.tile([2 * BL, 2 * BL], f32)
            make_identity(nc, ident8[:])
            ident1 = const_pool.tile([1, 1], f32)
            nc.gpsimd.memset(ident1[:], 1.0)

            # ---- load x, y and build contraction-layout xyT16 [128, k, 8] ----
            xy_in = persist.tile([2 * BL, D_IN], f32)
            nc.sync.dma_start(xy_in[0:BL, :], x_ext[:])
            nc.sync.dma_start(xy_in[BL : 2 * BL, :], y_ext[:])
            # funnel the two DMA-lane deps through one DVE op (ISA allows
            # only 2 sync waits per instruction)
            xy_sb = persist.tile([2 * BL, D_IN], f32)
            nc.vector.tensor_copy(xy_sb[:], xy_in[:])

            xyT16 = persist.tile([P, KC, 2 * BL], f16)
            for k in range(KC):
                ps = small_psum.tile([P, 2 * BL], f32, name="ps_xyT", tag="sp")
                nc.tensor.transpose(
                    ps[:], xy_sb[:, k * P : (k + 1) * P], ident8[:]
                )
                nc.scalar.copy(xyT16[:, k, :], ps[:])

            # ---- stream W: convert fp16, XBAR-transpose, fp16 matmuls ----
            # wt[p, ob, k, o_low] = W[ob*128 + o_low, k*128 + p]  (fp16)
            wt = persist.tile([P, OB, KC, P], f16)
            xy_proj = persist.tile([2 * BL, D_OUT], f32)  # rows 0-3 xp, 4-7 yp
            xpT2 = persist.tile([P, C2, 2, 2 * BL], f32)

            octx = ExitStack()
            wnat_pool = octx.enter_context(tc.tile_pool(name="wnat", bufs=2))
            w16_pool = octx.enter_context(tc.tile_pool(name="w16", bufs=2))
            mm_psum = octx.enter_context(
                tc.tile_pool(name="mm_psum", bufs=2, space="PSUM")
            )
            xpt_psum = octx.enter_context(
                tc.tile_pool(name="xpt_psum", bufs=2, space="PSUM")
            )
            for c in range(NCH):
                wnat = wnat_pool.tile([P, 4, D_IN], f32, name="wnat")
                nc.sync.dma_start(wnat[:], w_r[c])
                w16 = w16_pool.tile([P, 4, D_IN], f16, name="w16")
                nc.vector.tensor_copy(w16[:], wnat[:])
                for g in range(4):
                    ob = c * 4 + g
                    nc.sync.dma_start_transpose(wt[:, ob], w16[:, g, :])
                psxy = mm_psum.tile([2 * BL, 4 * P], f32, name="psxy")
                for k in range(KC):
                    nc.tensor.matmul(
                        psxy[:],
                        xyT16[:, k, :],
                        wt[:, c * 4 : (c + 1) * 4, k, :],
                        start=(k == 0),
                        stop=(k == KC - 1),
                    )
                nc.vector.tensor_copy(xy_proj[:, c * 512 : (c + 1) * 512], psxy[:])
                # incremental pair-packed transposes: xpT2[p, c2, jj, b]
                #   = xy_proj[b, c2*256 + 2p + jj]
                for c2 in (2 * c, 2 * c + 1):
                    pst = xpt_psum.tile([P, 2, 2 * BL], f32, name="pst")
                    for jj in range(2):
                        nc.tensor.transpose(
                            pst[:, jj, :],
                            xy_proj[:, c2 * 256 + jj : (c2 + 1) * 256 : 2],
                            ident8[:],
                        )
                    nc.scalar.copy(xpT2[:, c2], pst[:])

            if debug_stop == "proj":
                nc.sync.dma_start(dbg_proj[:], xy_proj[:])

            if debug_stop != "proj":
                # ---- norms: ss[r] = sum_o xy_proj[r, o]^2 (one fused DVE op) --
                sq_scratch = persist.tile([2 * BL, D_OUT], f32)
                ss = persist.tile([2 * BL, 1], f32)
                nc.vector.tensor_tensor(
                    sq_scratch[:], xy_proj[:], xy_proj[:], mybir.AluOpType.mult
                )
                nc.vector.reduce_sum(
                    ss[:], sq_scratch[:], axis=mybir.AxisListType.X
                )
                ps_ss = small_psum.tile([1, 2 * BL], f32, name="ps_ss", tag="sp")
                nc.tensor.transpose(ps_ss[:], ss[:], ident8[:])
                ssT = persist.tile([1, 2 * BL], f32)
                nc.vector.tensor_copy(ssT[:], ps_ss[:])

                # s_b = 1 / max(sqrt(ssx_b * ssy_b), eps), all on partition 0
                nprod = persist.tile([1, BL], f32)
                nc.vector.tensor_tensor(
                    nprod[:], ssT[:, 0:BL], ssT[:, BL : 2 * BL], mybir.AluOpType.mult
                )
                nsqrt = persist.tile([1, BL], f32)
                nc.scalar.sqrt(nsqrt[:], nprod[:])
                nmax = persist.tile([1, BL], f32)
                nc.vector.tensor_scalar_max(nmax[:], nsqrt[:], EPS)
                sT = persist.tile([1, BL], f32)
                nc.vector.reciprocal(sT[:], nmax[:])

                # place s_b onto partition BL+b (the yp rows of xy_proj)
                sdiag = persist.tile([1, 2 * BL], f32)
                nc.vector.memset(sdiag[:], 0.0)
                nc.vector.tensor_copy(sdiag[:, BL : 2 * BL], sT[:])
                ps_sc = small_psum.tile([2 * BL, 1], f32, name="ps_sc", tag="sp")
                nc.tensor.transpose(ps_sc[:], sdiag[:], ident1[:])
                scol = persist.tile([2 * BL, 1], f32)
                nc.scalar.copy(scol[:], ps_sc[:])

                # scaled fp16 yp rows, then DMA partition-broadcast per sample
                yps16 = persist.tile([P, D_OUT], f16)
                nc.vector.tensor_scalar_mul(
                    yps16[0 : 2 * BL, :],
                    xy_proj[0 : 2 * BL, :],
                    scol[0 : 2 * BL, 0:1],
                )
                dram_pool = octx.enter_context(
                    tc.tile_pool(name="dscratch", bufs=1, space="DRAM")
                )
                yps_dram = dram_pool.tile([2 * BL, D_OUT], f16)
                nc.sync.dma_start(yps_dram[:], yps16[0 : 2 * BL, :])
                ypb_pool = octx.enter_context(tc.tile_pool(name="ypb", bufs=1))
                ypb_tiles = []
                for b in range(BL):
                    ypb = ypb_pool.tile([P, D_OUT], f16, name=f"ypb{b}", tag=f"ypb{b}")
                    nc.scalar.dma_start(
                        ypb[:],
                        yps_dram[BL + b : BL + b + 1, :].to_broadcast((P, D_OUT)),
                    )
                    ypb_tiles.append(ypb)

                if debug_stop == "pre":
                    nc.sync.dma_start(dbg_xpt[:], xpT2[:])
                    for b in range(BL):
                        nc.sync.dma_start(dbg_ypb[b], ypb_tiles[b][:])

                if debug_stop is None:
                    # ---- outer products, 1MB fp16 tiles, stream out ----
                    out_pool = octx.enter_context(tc.tile_pool(name="outp", bufs=6))
                    idx = 0
                    for b in range(BL):
                        for c2 in range(C2):
                            ot = out_pool.tile([P, 2, D_OUT], f16, name="ot")
                            for jj in range(2):
                                if idx % 4 != 3:
                                    nc.vector.tensor_scalar_mul(
                                        ot[:, jj, :],
                                        ypb_tiles[b][:],
                                        xpT2[:, c2, jj, b : b + 1],
                                    )
                                else:
                                    nc.scalar.mul(
                                        ot[:, jj, :],
                                        ypb_tiles[b][:],
                                        xpT2[:, c2, jj, b : b + 1],
                                    )
                                idx += 1
                            nc.sync.dma_start(out_r[b, c2], ot[:])
            octx.close()

    nc.compile()
    return nc


def _get_nc():
    if "nc" not in _cache:
        _cache["nc"] = _build_nc()
    return _cache["nc"]


def kernel(x: np.ndarray, y: np.ndarray, W: np.ndarray) -> np.ndarray:
    from concourse.bass_utils import run_bass_kernel_spmd

    x = np.ascontiguousarray(x, dtype=np.float32)
    y = np.ascontiguousarray(y, dtype=np.float32)
    W = np.ascontiguousarray(W, dtype=np.float32)

    nc = _get_nc()
    in_maps = [
        {
            "x": np.ascontiguousarray(x[c * BL : (c + 1) * BL]),
            "y": np.ascontiguousarray(y[c * BL : (c + 1) * BL]),
            "W": W,
        }
        for c in range(NCORES)
    ]
    res = run_bass_kernel_spmd(nc, in_maps, list(range(NCORES))).results
    return np.concatenate(
        [res[c]["out"].astype(np.float32) for c in range(NCORES)], axis=0
    )


# revision 11
# speedup vs baseline: 1.2626x; 1.2626x over previous
"""Bilinear pooling kernel for Trainium2 (8 NeuronCores, data-parallel over batch).

reference:
    xp = x @ W.T          [B, 2048]
    yp = y @ W.T          [B, 2048]
    z[b] = flatten(outer(xp[b], yp[b]))    [B, 2048*2048]
    out = z / max(||z||_2, 1e-12)  (row-wise L2 normalize)

Key identity: ||outer(xp, yp)||_F = ||xp||_2 * ||yp||_2, so the normalizer is
computed from xp/yp directly. The 512MB output is written once, as float16
(rel err ~5e-4, well under the 2e-2 gate), halving HBM write traffic.

The replicated weight is pre-formatted host-side as W^T in fp16 (the
sharding hint's "1024x2048 weight"), so the device does no W transpose and
reads only 4MB of weights.

Per-core plan (4 samples each):
  1. DMA x,y shards; PE-transpose into xyT16 [128, 8, 8] fp16 (i on partitions).
  2. Stream W^T fp16 in 8 k-chunks [128, 2048]; fp16 matmuls accumulate
     xp/yp into 4 psum banks [8, 512] (one per o-quarter).
  3. Norms; s_b = 1/max(||xp_b||*||yp_b||, eps); scaled fp16 yp rows ->
     DRAM bounce -> partition-broadcast DMA to ypb[b] [128, 2048].
     PE transposes proj columns into xpT2 [128, c2, jj, 8] (pair-packed:
     o = c2*256 + 2p + jj).
  4. Outer products: ot[p, jj, :] = ypb[b] * xpT2[:, c2, jj, b] (DVE fp16);
     1MB DMAs out with 8KB contiguous per partition line.
"""

import sys
from contextlib import ExitStack

import numpy as np

if "/opt/trn_rl_repo" not in sys.path:
    sys.path.insert(0, "/opt/trn_rl_repo")

B, D_IN, D_OUT = 32, 1024, 2048
NCORES = 8
BL = B // NCORES  # 4 samples per core
P = 128
KC = D_IN // P  # 8 contraction chunks
C2 = 8  # output chunks per sample: o in [c2*256, (c2+1)*256), pair-packed
EPS = 1e-12

_cache = {}


def _build_nc(debug_stop=None):
    import concourse.bass as bass  # noqa: F401
    import concourse.mybir as mybir
    import concourse.tile as tile
    from concourse import bacc
    from concourse.masks import make_identity

    f32 = mybir.dt.float32
    f16 = mybir.dt.float16
    nc = bacc.Bacc()

    x_ext = nc.declare_dram_parameter("x", [BL, D_IN], f32, isOutput=False)
    y_ext = nc.declare_dram_parameter("y", [BL, D_IN], f32, isOutput=False)
    wt_ext = nc.declare_dram_parameter("wt", [D_IN, D_OUT], f16, isOutput=False)
    if debug_stop is None:
        out_ext = nc.declare_dram_parameter(
            "out", [BL, D_OUT * D_OUT], f16, isOutput=True
        )
        # out row b, flat index o*2048 + f with o = c2*256 + 2*p + jj
        out_r = out_ext[:].rearrange(
            "b (c2 p jj f) -> b c2 p jj f", c2=C2, p=P, jj=2, f=D_OUT
        )
    elif debug_stop == "proj":
        dbg_proj = nc.declare_dram_parameter(
            "dbg_proj", [2 * BL, D_OUT], f32, isOutput=True
        )
    elif debug_stop == "pre":
        dbg_xpt = nc.declare_dram_parameter(
            "dbg_xpt", [P, C2, 2, 2 * BL], f32, isOutput=True
        )
        dbg_ypb = nc.declare_dram_parameter(
            "dbg_ypb", [BL, P, D_OUT], f16, isOutput=True
        )

    # W^T row i = k*128 + p
    wt_r = wt_ext[:].rearrange("(k p) o -> k p o", k=KC, p=P)

    with tile.TileContext(nc) as tc:
        with (
            tc.tile_pool(name="const", bufs=1) as const_pool,
            tc.tile_pool(name="persist", bufs=1) as persist,
            tc.tile_pool(name="small_psum", bufs=2, space="PSUM") as small_psum,
            tc.tile_pool(name="mm_psum", bufs=1, space="PSUM") as mm_psum,
            tc.tile_pool(name="xpt_psum", bufs=2, space="PSUM") as xpt_psum,
        ):
            ident8 = const_pool.tile([2 * BL, 2 * BL], f32)
            make_identity(nc, ident8[:])
            ident1 = const_pool.tile([1, 1], f32)
            nc.gpsimd.memset(ident1[:], 1.0)

            # ---- stream W^T k-chunks (start these DMAs first) ----
            wt_all = persist.tile([P, KC, D_OUT], f16)
            for k in range(KC):
                nc.sync.dma_start(wt_all[:, k, :], wt_r[k])

            # ---- load x, y and build contraction-layout xyT16 [128, k, 8] ----
            xy_in = persist.tile([2 * BL, D_IN], f32)
            nc.sync.dma_start(xy_in[0:BL, :], x_ext[:])
            nc.sync.dma_start(xy_in[BL : 2 * BL, :], y_ext[:])
            # funnel the two DMA-lane deps through one DVE op (ISA allows
            # only 2 sync waits per instruction)
            xy_sb = persist.tile([2 * BL, D_IN], f32)
            nc.vector.tensor_copy(xy_sb[:], xy_in[:])

            xyT16 = persist.tile([P, KC, 2 * BL], f16)
            for k in range(KC):
                ps = small_psum.tile([P, 2 * BL], f32, name="ps_xyT", tag="sp")
                nc.tensor.transpose(
                    ps[:], xy_sb[:, k * P : (k + 1) * P], ident8[:]
                )
                nc.scalar.copy(xyT16[:, k, :], ps[:])

            # ---- projections: 4 psum banks (o-quarters), accumulate over k --
            psq = [
                mm_psum.tile([2 * BL, 512], f32, name=f"psq{oc}", tag=f"psq{oc}")
                for oc in range(4)
            ]
            for k in range(KC):
                for oc in range(4):
                    nc.tensor.matmul(
                        psq[oc][:],
                        xyT16[:, k, :],
                        wt_all[:, k, oc * 512 : (oc + 1) * 512],
                        start=(k == 0),
                        stop=(k == KC - 1),
                    )
            xy_proj = persist.tile([2 * BL, D_OUT], f32)  # rows 0-3 xp, 4-7 yp
            for oc in range(4):
                nc.vector.tensor_copy(
                    xy_proj[:, oc * 512 : (oc + 1) * 512], psq[oc][:]
                )

            # pair-packed transposes: xpT2[p, c2, jj, b] = xy_proj[b, c2*256+2p+jj]
            xpT2 = persist.tile([P, C2, 2, 2 * BL], f32)
            for c2 in range(C2):
                pst = xpt_psum.tile([P, 2, 2 * BL], f32, name="pst")
                for jj in range(2):
                    nc.tensor.transpose(
                        pst[:, jj, :],
                        xy_proj[:, c2 * 256 + jj : (c2 + 1) * 256 : 2],
                        ident8[:],
                    )
                nc.scalar.copy(xpT2[:, c2], pst[:])

            if debug_stop == "proj":
                nc.sync.dma_start(dbg_proj[:], xy_proj[:])

            octx = ExitStack()
            if debug_stop != "proj":
                # ---- norms: ss[r] = sum_o xy_proj[r, o]^2 ----
                sq_scratch = persist.tile([2 * BL, D_OUT], f32)
                ss = persist.tile([2 * BL, 1], f32)
                nc.vector.tensor_tensor(
                    sq_scratch[:], xy_proj[:], xy_proj[:], mybir.AluOpType.mult
                )
                nc.vector.reduce_sum(
                    ss[:], sq_scratch[:], axis=mybir.AxisListType.X
                )
                ps_ss = small_psum.tile([1, 2 * BL], f32, name="ps_ss", tag="sp")
                nc.tensor.transpose(ps_ss[:], ss[:], ident8[:])
                ssT = persist.tile([1, 2 * BL], f32)
                nc.vector.tensor_copy(ssT[:], ps_ss[:])

                # s_b = 1 / max(sqrt(ssx_b * ssy_b), eps), all on partition 0
                nprod = persist.tile([1, BL], f32)
                nc.vector.tensor_tensor(
                    nprod[:], ssT[:, 0:BL], ssT[:, BL : 2 * BL], mybir.AluOpType.mult
                )
                nsqrt = persist.tile([1, BL], f32)
                nc.scalar.sqrt(nsqrt[:], nprod[:])
                nmax = persist.tile([1, BL], f32)
                nc.vector.tensor_scalar_max(nmax[:], nsqrt[:], EPS)
                sT = persist.tile([1, BL], f32)
                nc.vector.reciprocal(sT[:], nmax[:])

                # place s_b onto partition BL+b (the yp rows of xy_proj)
                sdiag = persist.tile([1, 2 * BL], f32)
                nc.vector.memset(sdiag[:], 0.0)
                nc.vector.tensor_copy(sdiag[:, BL : 2 * BL], sT[:])
                ps_sc = small_psum.tile([2 * BL, 1], f32, name="ps_sc", tag="sp")
                nc.tensor.transpose(ps_sc[:], sdiag[:], ident1[:])
                scol = persist.tile([2 * BL, 1], f32)
                nc.scalar.copy(scol[:], ps_sc[:])

                # scaled fp16 yp rows -> DRAM bounce -> partition-broadcast
                yps16 = persist.tile([P, D_OUT], f16)
                nc.vector.tensor_scalar_mul(
                    yps16[0 : 2 * BL, :],
                    xy_proj[0 : 2 * BL, :],
                    scol[0 : 2 * BL, 0:1],
                )
                dram_pool = octx.enter_context(
                    tc.tile_pool(name="dscratch", bufs=1, space="DRAM")
                )
                yps_dram = dram_pool.tile([2 * BL, D_OUT], f16)
                nc.sync.dma_start(yps_dram[:], yps16[0 : 2 * BL, :])
                ypb_pool = octx.enter_context(tc.tile_pool(name="ypb", bufs=1))
                ypb_tiles = []
                for b in range(BL):
                    ypb = ypb_pool.tile([P, D_OUT], f16, name=f"ypb{b}", tag=f"ypb{b}")
                    nc.scalar.dma_start(
                        ypb[:],
                        yps_dram[BL + b : BL + b + 1, :].to_broadcast((P, D_OUT)),
                    )
                    ypb_tiles.append(ypb)

                if debug_stop == "pre":
                    nc.sync.dma_start(dbg_xpt[:], xpT2[:])
                    for b in range(BL):
                        nc.sync.dma_start(dbg_ypb[b], ypb_tiles[b][:])

                if debug_stop is None:
                    # ---- outer products, 1MB fp16 tiles, stream out ----
                    out_pool = octx.enter_context(tc.tile_pool(name="outp", bufs=8))
                    for b in range(BL):
                        for c2 in range(C2):
                            ot = out_pool.tile([P, 2, D_OUT], f16, name="ot")
                            for jj in range(2):
                                nc.vector.tensor_scalar_mul(
                                    ot[:, jj, :],
                                    ypb_tiles[b][:],
                                    xpT2[:, c2, jj, b : b + 1],
                                )
                            nc.sync.dma_start(out_r[b, c2], ot[:])
            octx.close()

    nc.compile()
    return nc


def _get_nc():
    if "nc" not in _cache:
        _cache["nc"] = _build_nc()
    return _cache["nc"]


def _in_maps(x, y, W):
    x = np.ascontiguousarray(x, dtype=np.float32)
    y = np.ascontiguousarray(y, dtype=np.float32)
    wt16 = np.ascontiguousarray(np.asarray(W, dtype=np.float32).T, dtype=np.float16)
    return [
        {
            "x": np.ascontiguousarray(x[c * BL : (c + 1) * BL]),
            "y": np.ascontiguousarray(y[c * BL : (c + 1) * BL]),
            "wt": wt16,
        }
        for c in range(NCORES)
    ]


def kernel(x: np.ndarray, y: np.ndarray, W: np.ndarray) -> np.ndarray:
    from concourse.bass_utils import run_bass_kernel_spmd

    nc = _get_nc()
    res = run_bass_kernel_spmd(nc, _in_maps(x, y, W), list(range(NCORES))).results
    return np.concatenate(
        [res[c]["out"].astype(np.float32) for c in range(NCORES)], axis=0
    )


# revision 12
# speedup vs baseline: 1.2852x; 1.0179x over previous
"""Bilinear pooling kernel for Trainium2 (8 NeuronCores, data-parallel over batch).

reference:
    xp = x @ W.T          [B, 2048]
    yp = y @ W.T          [B, 2048]
    z[b] = flatten(outer(xp[b], yp[b]))    [B, 2048*2048]
    out = z / max(||z||_2, 1e-12)  (row-wise L2 normalize)

Key identity: ||outer(xp, yp)||_F = ||xp||_2 * ||yp||_2, so the normalizer is
computed from xp/yp directly. The 512MB output is written once, as float16
(rel err ~5e-4, well under the 2e-2 gate), halving HBM write traffic.

The replicated weight is pre-formatted host-side as W^T in fp16 (the
sharding hint's "1024x2048 weight"), so the device does no W transpose and
reads only 4MB of weights.

Per-core plan (4 samples each):
  1. DMA x,y shards; PE-transpose into xyT16 [128, 8, 8] fp16 (i on partitions).
  2. Stream W^T fp16 in 8 k-chunks [128, 2048]; fp16 matmuls accumulate
     xp/yp into 4 psum banks [8, 512] (one per o-quarter).
  3. Norms; s_b = 1/max(||xp_b||*||yp_b||, eps); scaled fp16 yp rows ->
     DRAM bounce -> partition-broadcast DMA to ypb[b] [128, 2048].
     PE transposes proj columns into xpT2 [128, c2, jj, 8] (pair-packed:
     o = c2*256 + 2p + jj).
  4. Outer products: ot[p, jj, :] = ypb[b] * xpT2[:, c2, jj, b] (DVE fp16);
     1MB DMAs out with 8KB contiguous per partition line.
"""

import sys
from contextlib import ExitStack

import numpy as np

if "/opt/trn_rl_repo" not in sys.path:
    sys.path.insert(0, "/opt/trn_rl_repo")

B, D_IN, D_OUT = 32, 1024, 2048
NCORES = 8
BL = B // NCORES  # 4 samples per core
P = 128
KC = D_IN // P  # 8 contraction chunks
C2 = 8  # output chunks per sample: o in [c2*256, (c2+1)*256), pair-packed
EPS = 1e-12

_cache = {}


def _build_nc(debug_stop=None):
    import concourse.bass as bass  # noqa: F401
    import concourse.mybir as mybir
    import concourse.tile as tile
    from concourse import bacc
    from concourse.masks import make_identity

    f32 = mybir.dt.float32
    f16 = mybir.dt.float16
    nc = bacc.Bacc()

    x_ext = nc.declare_dram_parameter("x", [BL, D_IN], f32, isOutput=False)
    y_ext = nc.declare_dram_parameter("y", [BL, D_IN], f32, isOutput=False)
    wt_ext = nc.declare_dram_parameter("wt", [D_IN, D_OUT], f16, isOutput=False)
    if debug_stop is None:
        out_ext = nc.declare_dram_parameter(
            "out", [BL, D_OUT * D_OUT], f16, isOutput=True
        )
        # out row b, flat index o*2048 + f with o = c2*256 + 2*p + jj
        out_r = out_ext[:].rearrange(
            "b (c2 p jj f) -> b c2 p jj f", c2=C2, p=P, jj=2, f=D_OUT
        )
    elif debug_stop == "proj":
        dbg_proj = nc.declare_dram_parameter(
            "dbg_proj", [2 * BL, D_OUT], f32, isOutput=True
        )
    elif debug_stop == "pre":
        dbg_xpt = nc.declare_dram_parameter(
            "dbg_xpt", [P, C2, 2, 2 * BL], f32, isOutput=True
        )
        dbg_ypb = nc.declare_dram_parameter(
            "dbg_ypb", [BL, P, D_OUT], f16, isOutput=True
        )

    # W^T row i = k*128 + p
    wt_r = wt_ext[:].rearrange("(k p) o -> k p o", k=KC, p=P)

    with tile.TileContext(nc) as tc:
        with (
            tc.tile_pool(name="const", bufs=1) as const_pool,
            tc.tile_pool(name="persist", bufs=1) as persist,
            tc.tile_pool(name="small_psum", bufs=2, space="PSUM") as small_psum,
            tc.tile_pool(name="mm_psum", bufs=1, space="PSUM") as mm_psum,
            tc.tile_pool(name="xpt_psum", bufs=2, space="PSUM") as xpt_psum,
        ):
            ident8 = const_pool.tile([2 * BL, 2 * BL], f32)
            make_identity(nc, ident8[:])
            ident1 = const_pool.tile([1, 1], f32)
            nc.gpsimd.memset(ident1[:], 1.0)

            # ---- load x, y first (tiny; must not queue behind the 4MB W^T) --
            xy_in = persist.tile([2 * BL, D_IN], f32)
            nc.sync.dma_start(xy_in[0:BL, :], x_ext[:])
            nc.sync.dma_start(xy_in[BL : 2 * BL, :], y_ext[:])

            # ---- stream W^T k-chunks ----
            wt_all = persist.tile([P, KC, D_OUT], f16)
            for k in range(KC):
                nc.sync.dma_start(wt_all[:, k, :], wt_r[k])
            # funnel the two DMA-lane deps through one DVE op (ISA allows
            # only 2 sync waits per instruction)
            xy_sb = persist.tile([2 * BL, D_IN], f32)
            nc.vector.tensor_copy(xy_sb[:], xy_in[:])

            xyT16 = persist.tile([P, KC, 2 * BL], f16)
            for k in range(KC):
                ps = small_psum.tile([P, 2 * BL], f32, name="ps_xyT", tag="sp")
                nc.tensor.transpose(
                    ps[:], xy_sb[:, k * P : (k + 1) * P], ident8[:]
                )
                nc.scalar.copy(xyT16[:, k, :], ps[:])

            # ---- projections: 4 psum banks (o-quarters), accumulate over k --
            psq = [
                mm_psum.tile([2 * BL, 512], f32, name=f"psq{oc}", tag=f"psq{oc}")
                for oc in range(4)
            ]
            for k in range(KC):
                for oc in range(4):
                    nc.tensor.matmul(
                        psq[oc][:],
                        xyT16[:, k, :],
                        wt_all[:, k, oc * 512 : (oc + 1) * 512],
                        start=(k == 0),
                        stop=(k == KC - 1),
                    )
            xy_proj = persist.tile([2 * BL, D_OUT], f32)  # rows 0-3 xp, 4-7 yp
            for oc in range(4):
                nc.vector.tensor_copy(
                    xy_proj[:, oc * 512 : (oc + 1) * 512], psq[oc][:]
                )

            # pair-packed transposes: xpT2[p, c2, jj, b] = xy_proj[b, c2*256+2p+jj]
            xpT2 = persist.tile([P, C2, 2, 2 * BL], f32)
            for c2 in range(C2):
                pst = xpt_psum.tile([P, 2, 2 * BL], f32, name="pst")
                for jj in range(2):
                    nc.tensor.transpose(
                        pst[:, jj, :],
                        xy_proj[:, c2 * 256 + jj : (c2 + 1) * 256 : 2],
                        ident8[:],
                    )
                nc.scalar.copy(xpT2[:, c2], pst[:])

            if debug_stop == "proj":
                nc.sync.dma_start(dbg_proj[:], xy_proj[:])

            octx = ExitStack()
            if debug_stop != "proj":
                # ---- norms: ss[r] = sum_o xy_proj[r, o]^2 ----
                sq_scratch = persist.tile([2 * BL, D_OUT], f32)
                ss = persist.tile([2 * BL, 1], f32)
                nc.vector.tensor_tensor(
                    sq_scratch[:], xy_proj[:], xy_proj[:], mybir.AluOpType.mult
                )
                nc.vector.reduce_sum(
                    ss[:], sq_scratch[:], axis=mybir.AxisListType.X
                )
                ps_ss = small_psum.tile([1, 2 * BL], f32, name="ps_ss", tag="sp")
                nc.tensor.transpose(ps_ss[:], ss[:], ident8[:])
                ssT = persist.tile([1, 2 * BL], f32)
                nc.vector.tensor_copy(ssT[:], ps_ss[:])

                # s_b = 1 / max(sqrt(ssx_b * ssy_b), eps), all on partition 0
                nprod = persist.tile([1, BL], f32)
                nc.vector.tensor_tensor(
                    nprod[:], ssT[:, 0:BL], ssT[:, BL : 2 * BL], mybir.AluOpType.mult
                )
                nsqrt = persist.tile([1, BL], f32)
                nc.scalar.sqrt(nsqrt[:], nprod[:])
                nmax = persist.tile([1, BL], f32)
                nc.vector.tensor_scalar_max(nmax[:], nsqrt[:], EPS)
                sT = persist.tile([1, BL], f32)
                nc.vector.reciprocal(sT[:], nmax[:])

                # place s_b onto partition BL+b (the yp rows of xy_proj)
                sdiag = persist.tile([1, 2 * BL], f32)
                nc.vector.memset(sdiag[:], 0.0)
                nc.vector.tensor_copy(sdiag[:, BL : 2 * BL], sT[:])
                ps_sc = small_psum.tile([2 * BL, 1], f32, name="ps_sc", tag="sp")
                nc.tensor.transpose(ps_sc[:], sdiag[:], ident1[:])
                scol = persist.tile([2 * BL, 1], f32)
                nc.scalar.copy(scol[:], ps_sc[:])

                # scaled fp16 yp rows -> DRAM bounce -> partition-broadcast
                yps16 = persist.tile([P, D_OUT], f16)
                nc.vector.tensor_scalar_mul(
                    yps16[0 : 2 * BL, :],
                    xy_proj[0 : 2 * BL, :],
                    scol[0 : 2 * BL, 0:1],
                )
                dram_pool = octx.enter_context(
                    tc.tile_pool(name="dscratch", bufs=1, space="DRAM")
                )
                yps_dram = dram_pool.tile([2 * BL, D_OUT], f16)
                nc.sync.dma_start(yps_dram[:], yps16[0 : 2 * BL, :])
                ypb_pool = octx.enter_context(tc.tile_pool(name="ypb", bufs=1))
                ypb_tiles = []
                for b in range(BL):
                    ypb = ypb_pool.tile([P, D_OUT], f16, name=f"ypb{b}", tag=f"ypb{b}")
                    nc.scalar.dma_start(
                        ypb[:],
                        yps_dram[BL + b : BL + b + 1, :].to_broadcast((P, D_OUT)),
                    )
                    ypb_tiles.append(ypb)

                if debug_stop == "pre":
                    nc.sync.dma_start(dbg_xpt[:], xpT2[:])
                    for b in range(BL):
                        nc.sync.dma_start(dbg_ypb[b], ypb_tiles[b][:])

                if debug_stop is None:
                    # ---- outer products, 1MB fp16 tiles, stream out ----
                    out_pool = octx.enter_context(tc.tile_pool(name="outp", bufs=12))
                    for b in range(BL):
                        for c2 in range(C2):
                            ot = out_pool.tile([P, 2, D_OUT], f16, name="ot")
                            for jj in range(2):
                                nc.vector.tensor_scalar_mul(
                                    ot[:, jj, :],
                                    ypb_tiles[b][:],
                                    xpT2[:, c2, jj, b : b + 1],
                                )
                            nc.sync.dma_start(out_r[b, c2], ot[:])
            octx.close()

    nc.compile()
    return nc


def _get_nc():
    if "nc" not in _cache:
        _cache["nc"] = _build_nc()
    return _cache["nc"]


def _in_maps(x, y, W):
    x = np.ascontiguousarray(x, dtype=np.float32)
    y = np.ascontiguousarray(y, dtype=np.float32)
    wt16 = np.ascontiguousarray(np.asarray(W, dtype=np.float32).T, dtype=np.float16)
    return [
        {
            "x": np.ascontiguousarray(x[c * BL : (c + 1) * BL]),
            "y": np.ascontiguousarray(y[c * BL : (c + 1) * BL]),
            "wt": wt16,
        }
        for c in range(NCORES)
    ]


def kernel(x: np.ndarray, y: np.ndarray, W: np.ndarray) -> np.ndarray:
    from concourse.bass_utils import run_bass_kernel_spmd

    nc = _get_nc()
    res = run_bass_kernel_spmd(nc, _in_maps(x, y, W), list(range(NCORES))).results
    return np.concatenate(
        [res[c]["out"].astype(np.float32) for c in range(NCORES)], axis=0
    )


# revision 14
# speedup vs baseline: 1.5230x; 1.1851x over previous
"""Bilinear pooling kernel for Trainium2 (8 NeuronCores, data-parallel over batch).

reference:
    xp = x @ W.T          [B, 2048]
    yp = y @ W.T          [B, 2048]
    z[b] = flatten(outer(xp[b], yp[b]))    [B, 2048*2048]
    out = z / max(||z||_2, 1e-12)  (row-wise L2 normalize)

Key identity: ||outer(xp, yp)||_F = ||xp||_2 * ||yp||_2, so the normalizer is
computed from xp/yp directly. The 512MB output is written once, as float16
(rel err ~5e-4, well under the 2e-2 gate), halving HBM write traffic.

The replicated weight is pre-formatted host-side as W^T in fp16 (the
sharding hint's "1024x2048 weight"), so the device does no W transpose and
reads only 4MB of weights.

Per-core plan (4 samples each):
  1. DMA x,y shards; PE-transpose into xyT16 [128, 8, 8] fp16 (i on partitions).
  2. Stream W^T fp16 in 8 k-chunks [128, 2048]; fp16 matmuls accumulate
     xp/yp into 4 psum banks [8, 512] (one per o-quarter).
  3. Norms; s_b = 1/max(||xp_b||*||yp_b||, eps); scaled fp16 yp rows ->
     DRAM bounce -> partition-broadcast DMA to ypb[b] [128, 2048].
     PE transposes proj columns into xpT2 [128, c2, jj, 8] (pair-packed:
     o = c2*256 + 2p + jj).
  4. Outer products: ot[p, jj, :] = ypb[b] * xpT2[:, c2, jj, b] (DVE fp16);
     1MB DMAs out with 8KB contiguous per partition line.
"""

import sys
from contextlib import ExitStack

import numpy as np

if "/opt/trn_rl_repo" not in sys.path:
    sys.path.insert(0, "/opt/trn_rl_repo")

B, D_IN, D_OUT = 32, 1024, 2048
NCORES = 8
BL = B // NCORES  # 4 samples per core
P = 128
KC = D_IN // P  # 8 contraction chunks
C2 = 8  # output chunks per sample: o in [c2*256, (c2+1)*256), pair-packed
EPS = 1e-12

_cache = {}


def _build_nc(debug_stop=None):
    import concourse.bass as bass  # noqa: F401
    import concourse.mybir as mybir
    import concourse.tile as tile
    from concourse import bacc

    f32 = mybir.dt.float32
    f16 = mybir.dt.float16
    nc = bacc.Bacc()

    x_ext = nc.declare_dram_parameter("x", [BL, D_IN], f32, isOutput=False)
    y_ext = nc.declare_dram_parameter("y", [BL, D_IN], f32, isOutput=False)
    wt_ext = nc.declare_dram_parameter("wt", [D_IN, D_OUT], f16, isOutput=False)
    eye8_ext = nc.declare_dram_parameter("eye8", [2 * BL, 2 * BL], f32, isOutput=False)
    if debug_stop is None:
        out_ext = nc.declare_dram_parameter(
            "out", [BL, D_OUT * D_OUT], f16, isOutput=True
        )
        # out row b, flat index o*2048 + f with o = c2*256 + 2*p + jj
        out_r = out_ext[:].rearrange(
            "b (c2 p jj f) -> b c2 p jj f", c2=C2, p=P, jj=2, f=D_OUT
        )
    elif debug_stop == "proj":
        dbg_proj = nc.declare_dram_parameter(
            "dbg_proj", [2 * BL, D_OUT], f32, isOutput=True
        )
    elif debug_stop == "pre":
        dbg_xpt = nc.declare_dram_parameter(
            "dbg_xpt", [P, C2, 2, 2 * BL], f32, isOutput=True
        )
        dbg_ypb = nc.declare_dram_parameter(
            "dbg_ypb", [BL, P, D_OUT], f16, isOutput=True
        )

    # W^T row i = k*128 + p
    wt_r = wt_ext[:].rearrange("(k p) o -> k p o", k=KC, p=P)

    with tile.TileContext(nc) as tc:
        with (
            tc.tile_pool(name="const", bufs=1) as const_pool,
            tc.tile_pool(name="persist", bufs=1) as persist,
            tc.tile_pool(name="small_psum", bufs=2, space="PSUM") as small_psum,
            tc.tile_pool(name="mm_psum", bufs=1, space="PSUM") as mm_psum,
            tc.tile_pool(name="xpt_psum", bufs=2, space="PSUM") as xpt_psum,
        ):
            ident8 = const_pool.tile([2 * BL, 2 * BL], f32)
            nc.sync.dma_start(ident8[:], eye8_ext[:])
            ident1 = const_pool.tile([1, 1], f32)
            nc.vector.memset(ident1[:], 1.0)

            # ---- load x, y first (tiny; must not queue behind the 4MB W^T) --
            xy_in = persist.tile([2 * BL, D_IN], f32)
            nc.sync.dma_start(xy_in[0:BL, :], x_ext[:])
            nc.sync.dma_start(xy_in[BL : 2 * BL, :], y_ext[:])

            # ---- stream W^T k-chunks ----
            wt_all = persist.tile([P, KC, D_OUT], f16)
            for k in range(KC):
                nc.sync.dma_start(wt_all[:, k, :], wt_r[k])
            # funnel the two DMA-lane deps through one DVE op (ISA allows
            # only 2 sync waits per instruction)
            xy_sb = persist.tile([2 * BL, D_IN], f32)
            nc.vector.tensor_copy(xy_sb[:], xy_in[:])

            xyT16 = persist.tile([P, KC, 2 * BL], f16)
            for k in range(KC):
                ps = small_psum.tile([P, 2 * BL], f32, name="ps_xyT", tag="sp")
                nc.tensor.transpose(
                    ps[:], xy_sb[:, k * P : (k + 1) * P], ident8[:]
                )
                nc.scalar.copy(xyT16[:, k, :], ps[:])

            # ---- projections: 4 psum banks (o-quarters), accumulate over k --
            psq = [
                mm_psum.tile([2 * BL, 512], f32, name=f"psq{oc}", tag=f"psq{oc}")
                for oc in range(4)
            ]
            for k in range(KC):
                for oc in range(4):
                    nc.tensor.matmul(
                        psq[oc][:],
                        xyT16[:, k, :],
                        wt_all[:, k, oc * 512 : (oc + 1) * 512],
                        start=(k == 0),
                        stop=(k == KC - 1),
                    )
            xy_proj = persist.tile([2 * BL, D_OUT], f32)  # rows 0-3 xp, 4-7 yp
            for oc in range(4):
                nc.vector.tensor_copy(
                    xy_proj[:, oc * 512 : (oc + 1) * 512], psq[oc][:]
                )

            # pair-packed transposes: xpT2[p, c2, jj, b] = xy_proj[b, c2*256+2p+jj]
            xpT2 = persist.tile([P, C2, 2, 2 * BL], f32)
            for c2 in range(C2):
                pst = xpt_psum.tile([P, 2, 2 * BL], f32, name="pst")
                for jj in range(2):
                    nc.tensor.transpose(
                        pst[:, jj, :],
                        xy_proj[:, c2 * 256 + jj : (c2 + 1) * 256 : 2],
                        ident8[:],
                    )
                nc.scalar.copy(xpT2[:, c2], pst[:])

            if debug_stop == "proj":
                nc.sync.dma_start(dbg_proj[:], xy_proj[:])

            octx = ExitStack()
            if debug_stop != "proj":
                # ---- norms: ss[r] = sum_o xy_proj[r, o]^2 (ACT Square+accum
                # per o-quarter straight from psum, then tiny column add) ----
                sq_scratch = persist.tile([2 * BL, D_OUT], f32)
                ss4 = persist.tile([2 * BL, 4], f32)
                ss = persist.tile([2 * BL, 1], f32)
                for oc in range(4):
                    nc.scalar.activation(
                        sq_scratch[:, oc * 512 : (oc + 1) * 512],
                        psq[oc][:],
                        mybir.ActivationFunctionType.Square,
                        accum_out=ss4[:, oc : oc + 1],
                    )
                nc.vector.reduce_sum(ss[:], ss4[:], axis=mybir.AxisListType.X)
                ps_ss = small_psum.tile([1, 2 * BL], f32, name="ps_ss", tag="sp")
                nc.tensor.transpose(ps_ss[:], ss[:], ident8[:])
                ssT = persist.tile([1, 2 * BL], f32)
                nc.vector.tensor_copy(ssT[:], ps_ss[:])

                # s_b = 1 / max(sqrt(ssx_b * ssy_b), eps), all on partition 0
                nprod = persist.tile([1, BL], f32)
                nc.vector.tensor_tensor(
                    nprod[:], ssT[:, 0:BL], ssT[:, BL : 2 * BL], mybir.AluOpType.mult
                )
                nsqrt = persist.tile([1, BL], f32)
                nc.scalar.sqrt(nsqrt[:], nprod[:])
                nmax = persist.tile([1, BL], f32)
                nc.vector.tensor_scalar_max(nmax[:], nsqrt[:], EPS)
                sT = persist.tile([1, BL], f32)
                nc.vector.reciprocal(sT[:], nmax[:])

                # place s_b onto partition BL+b (the yp rows of xy_proj)
                sdiag = persist.tile([1, 2 * BL], f32)
                nc.vector.memset(sdiag[:], 0.0)
                nc.vector.tensor_copy(sdiag[:, BL : 2 * BL], sT[:])
                ps_sc = small_psum.tile([2 * BL, 1], f32, name="ps_sc", tag="sp")
                nc.tensor.transpose(ps_sc[:], sdiag[:], ident1[:])
                scol = persist.tile([2 * BL, 1], f32)
                nc.scalar.copy(scol[:], ps_sc[:])

                # scaled fp16 yp rows -> DRAM bounce -> partition-broadcast
                yps16 = persist.tile([P, D_OUT], f16)
                nc.vector.tensor_scalar_mul(
                    yps16[0 : 2 * BL, :],
                    xy_proj[0 : 2 * BL, :],
                    scol[0 : 2 * BL, 0:1],
                )
                dram_pool = octx.enter_context(
                    tc.tile_pool(name="dscratch", bufs=1, space="DRAM")
                )
                yps_dram = dram_pool.tile([2 * BL, D_OUT], f16)
                nc.sync.dma_start(yps_dram[:], yps16[0 : 2 * BL, :])
                ypb_pool = octx.enter_context(tc.tile_pool(name="ypb", bufs=1))
                ypb_tiles = []
                for b in range(BL):
                    ypb = ypb_pool.tile([P, D_OUT], f16, name=f"ypb{b}", tag=f"ypb{b}")
                    for q in range(4):
                        nc.scalar.dma_start(
                            ypb[q * 32 : (q + 1) * 32, :],
                            yps_dram[BL + b : BL + b + 1, :].to_broadcast(
                                (32, D_OUT)
                            ),
                        )
                    ypb_tiles.append(ypb)

                if debug_stop == "pre":
                    nc.sync.dma_start(dbg_xpt[:], xpT2[:])
                    for b in range(BL):
                        nc.sync.dma_start(dbg_ypb[b], ypb_tiles[b][:])

                if debug_stop is None:
                    # ---- outer products, 1MB fp16 tiles, stream out ----
                    out_pool = octx.enter_context(tc.tile_pool(name="outp", bufs=12))
                    for b in range(BL):
                        for c2 in range(C2):
                            ot = out_pool.tile([P, 2, D_OUT], f16, name="ot")
                            for jj in range(2):
                                nc.vector.tensor_scalar_mul(
                                    ot[:, jj, :],
                                    ypb_tiles[b][:],
                                    xpT2[:, c2, jj, b : b + 1],
                                )
                            nc.sync.dma_start(out_r[b, c2], ot[:])
            octx.close()

    nc.compile()
    return nc


def _get_nc():
    if "nc" not in _cache:
        _cache["nc"] = _build_nc()
    return _cache["nc"]


def _in_maps(x, y, W):
    x = np.ascontiguousarray(x, dtype=np.float32)
    y = np.ascontiguousarray(y, dtype=np.float32)
    wt16 = np.ascontiguousarray(np.asarray(W, dtype=np.float32).T, dtype=np.float16)
    return [
        {
            "x": np.ascontiguousarray(x[c * BL : (c + 1) * BL]),
            "y": np.ascontiguousarray(y[c * BL : (c + 1) * BL]),
            "wt": wt16,
            "eye8": np.eye(2 * BL, dtype=np.float32),
        }
        for c in range(NCORES)
    ]


def kernel(x: np.ndarray, y: np.ndarray, W: np.ndarray) -> np.ndarray:
    from concourse.bass_utils import run_bass_kernel_spmd

    nc = _get_nc()
    res = run_bass_kernel_spmd(nc, _in_maps(x, y, W), list(range(NCORES))).results
    return np.concatenate(
        [res[c]["out"].astype(np.float32) for c in range(NCORES)], axis=0
    )


# revision 17
# speedup vs baseline: 1.5960x; 1.0479x over previous
"""Bilinear pooling kernel for Trainium2 (8 NeuronCores, data-parallel over batch).

reference:
    xp = x @ W.T          [B, 2048]
    yp = y @ W.T          [B, 2048]
    z[b] = flatten(outer(xp[b], yp[b]))    [B, 2048*2048]
    out = z / max(||z||_2, 1e-12)  (row-wise L2 normalize)

Key identity: ||outer(xp, yp)||_F = ||xp||_2 * ||yp||_2, so the normalizer is
computed from xp/yp directly. The 512MB output is written once, as float16
(rel err ~5e-4, well under the 2e-2 gate), halving HBM write traffic.

The replicated weight is pre-formatted host-side as W^T in fp16 (the
sharding hint's "1024x2048 weight"), so the device does no W transpose and
reads only 4MB of weights.

Per-core plan (4 samples each):
  1. DMA x,y shards; PE-transpose into xyT16 [128, 8, 8] fp16 (i on partitions).
  2. Stream W^T fp16 in 8 k-chunks [128, 2048]; fp16 matmuls accumulate
     xp/yp into 4 psum banks [8, 512] (one per o-quarter).
  3. Norms; s_b = 1/max(||xp_b||*||yp_b||, eps); scaled fp16 yp rows ->
     DRAM bounce -> partition-broadcast DMA to ypb[b] [128, 2048].
     PE transposes proj columns into xpT2 [128, c2, jj, 8] (pair-packed:
     o = c2*256 + 2p + jj).
  4. Outer products: ot[p, jj, :] = ypb[b] * xpT2[:, c2, jj, b] (DVE fp16);
     1MB DMAs out with 8KB contiguous per partition line.
"""

import sys
from contextlib import ExitStack

import numpy as np

if "/opt/trn_rl_repo" not in sys.path:
    sys.path.insert(0, "/opt/trn_rl_repo")

B, D_IN, D_OUT = 32, 1024, 2048
NCORES = 8
BL = B // NCORES  # 4 samples per core
P = 128
KC = D_IN // P  # 8 contraction chunks
C2 = 8  # output chunks per sample: o in [c2*256, (c2+1)*256), pair-packed
EPS = 1e-12

_cache = {}


def _build_nc(debug_stop=None):
    import concourse.bass as bass  # noqa: F401
    import concourse.mybir as mybir
    import concourse.tile as tile
    from concourse import bacc

    f32 = mybir.dt.float32
    f16 = mybir.dt.float16
    nc = bacc.Bacc()

    xyt_ext = nc.declare_dram_parameter("xyt16", [P, KC, 2 * BL], f16, isOutput=False)
    wt_ext = nc.declare_dram_parameter("wt", [D_IN, D_OUT], f16, isOutput=False)
    eye8_ext = nc.declare_dram_parameter("eye8", [2 * BL, 2 * BL], f32, isOutput=False)
    mask_ext = nc.declare_dram_parameter("mask16", [2 * BL, BL, P], f16, isOutput=False)
    if debug_stop is None:
        out_ext = nc.declare_dram_parameter(
            "out", [BL, D_OUT * D_OUT], f16, isOutput=True
        )
        # out row b, flat index o*2048 + f with o = c2*256 + 2*p + jj
        out_r = out_ext[:].rearrange(
            "b (c2 p jj f) -> b c2 p jj f", c2=C2, p=P, jj=2, f=D_OUT
        )
    elif debug_stop == "proj":
        dbg_proj = nc.declare_dram_parameter(
            "dbg_proj", [2 * BL, D_OUT], f32, isOutput=True
        )
    elif debug_stop == "pre":
        dbg_xpt = nc.declare_dram_parameter(
            "dbg_xpt", [P, C2, 2, 2 * BL], f32, isOutput=True
        )
        dbg_ypb = nc.declare_dram_parameter(
            "dbg_ypb", [BL, P, D_OUT], f16, isOutput=True
        )

    # W^T row i = k*128 + p
    wt_r = wt_ext[:].rearrange("(k p) o -> k p o", k=KC, p=P)

    with tile.TileContext(nc) as tc:
        with (
            tc.tile_pool(name="const", bufs=1) as const_pool,
            tc.tile_pool(name="persist", bufs=1) as persist,
            tc.tile_pool(name="small_psum", bufs=2, space="PSUM") as small_psum,
            tc.tile_pool(name="mm_psum", bufs=1, space="PSUM") as mm_psum,
            tc.tile_pool(name="bcast_psum", bufs=2, space="PSUM") as bcast_psum,
        ):
            ident8 = const_pool.tile([2 * BL, 2 * BL], f32)
            nc.sync.dma_start(ident8[:], eye8_ext[:])
            ident1 = const_pool.tile([1, 1], f32)
            nc.vector.memset(ident1[:], 1.0)
            mask16 = const_pool.tile([2 * BL, BL, P], f16)
            nc.sync.dma_start(mask16[:], mask_ext[:])

            # ---- load xyT16 first (tiny; must not queue behind the 4MB W^T) --
            xyT16 = persist.tile([P, KC, 2 * BL], f16)
            nc.sync.dma_start(xyT16[:], xyt_ext[:])

            # ---- stream W^T k-chunks ----
            wt_all = persist.tile([P, KC, D_OUT], f16)
            for k in range(KC):
                nc.sync.dma_start(wt_all[:, k, :], wt_r[k])
            # ---- projections: 4 psum banks (o-quarters), accumulate over k --
            psq = [
                mm_psum.tile([2 * BL, 512], f32, name=f"psq{oc}", tag=f"psq{oc}")
                for oc in range(4)
            ]
            for k in range(KC):
                for oc in range(4):
                    nc.tensor.matmul(
                        psq[oc][:],
                        xyT16[:, k, :],
                        wt_all[:, k, oc * 512 : (oc + 1) * 512],
                        start=(k == 0),
                        stop=(k == KC - 1),
                    )
            xy_proj = persist.tile([2 * BL, D_OUT], f32)  # rows 0-3 xp, 4-7 yp
            for oc in range(4):
                nc.vector.tensor_copy(
                    xy_proj[:, oc * 512 : (oc + 1) * 512], psq[oc][:]
                )

            # pair-packed transposes: xpT2[p, c2, jj, b] = xy_proj[b, c2*256+2p+jj]
            xpT2 = persist.tile([P, C2, 2, 2 * BL], f32)
            for c2 in range(C2):
                pst = small_psum.tile([P, 2, 2 * BL], f32, name="pst", tag="sp")
                for jj in range(2):
                    nc.tensor.transpose(
                        pst[:, jj, :],
                        xy_proj[:, c2 * 256 + jj : (c2 + 1) * 256 : 2],
                        ident8[:],
                    )
                nc.scalar.copy(xpT2[:, c2], pst[:])

            if debug_stop == "proj":
                nc.sync.dma_start(dbg_proj[:], xy_proj[:])

            octx = ExitStack()
            if debug_stop != "proj":
                # ---- norms: ss[r] = sum_o xy_proj[r, o]^2 (ACT Square+accum
                # per o-quarter straight from psum, then tiny column add) ----
                sq_scratch = persist.tile([2 * BL, D_OUT], f32)
                ss4 = persist.tile([2 * BL, 4], f32)
                ss = persist.tile([2 * BL, 1], f32)
                for oc in range(4):
                    nc.scalar.activation(
                        sq_scratch[:, oc * 512 : (oc + 1) * 512],
                        psq[oc][:],
                        mybir.ActivationFunctionType.Square,
                        accum_out=ss4[:, oc : oc + 1],
                    )
                nc.vector.reduce_sum(ss[:], ss4[:], axis=mybir.AxisListType.X)
                ps_ss = small_psum.tile([1, 2 * BL], f32, name="ps_ss", tag="sp")
                nc.tensor.transpose(ps_ss[:], ss[:], ident8[:])
                ssT = persist.tile([1, 2 * BL], f32)
                nc.vector.tensor_copy(ssT[:], ps_ss[:])

                # s_b = 1 / max(sqrt(ssx_b * ssy_b), eps), all on partition 0
                nprod = persist.tile([1, BL], f32)
                nc.vector.tensor_tensor(
                    nprod[:], ssT[:, 0:BL], ssT[:, BL : 2 * BL], mybir.AluOpType.mult
                )
                nsqrt = persist.tile([1, BL], f32)
                nc.scalar.sqrt(nsqrt[:], nprod[:])
                nmax = persist.tile([1, BL], f32)
                nc.vector.tensor_scalar_max(nmax[:], nsqrt[:], EPS)
                sT = persist.tile([1, BL], f32)
                nc.vector.reciprocal(sT[:], nmax[:])

                # place s_b onto partition BL+b (the yp rows of xy_proj)
                sdiag = persist.tile([1, 2 * BL], f32)
                nc.vector.memset(sdiag[:], 0.0)
                nc.vector.tensor_copy(sdiag[:, BL : 2 * BL], sT[:])
                ps_sc = small_psum.tile([2 * BL, 1], f32, name="ps_sc", tag="sp")
                nc.tensor.transpose(ps_sc[:], sdiag[:], ident1[:])
                scol = persist.tile([2 * BL, 1], f32)
                nc.scalar.copy(scol[:], ps_sc[:])

                # scaled fp16 yp rows -> DRAM bounce -> partition-broadcast
                yps16 = persist.tile([P, D_OUT], f16)
                nc.vector.tensor_scalar_mul(
                    yps16[0 : 2 * BL, :],
                    xy_proj[0 : 2 * BL, :],
                    scol[0 : 2 * BL, 0:1],
                )
                ypb_pool = octx.enter_context(tc.tile_pool(name="ypb", bufs=1))
                ypb_tiles = []
                for b in range(BL):
                    ypb = ypb_pool.tile([P, D_OUT], f16, name=f"ypb{b}", tag=f"ypb{b}")
                    for n2 in range(4):
                        psb = bcast_psum.tile([P, 512], f32, name="psb", tag="psb")
                        nc.tensor.matmul(
                            psb[:],
                            mask16[:, b, :],
                            yps16[0 : 2 * BL, n2 * 512 : (n2 + 1) * 512],
                            start=True,
                            stop=True,
                        )
                        nc.scalar.copy(ypb[:, n2 * 512 : (n2 + 1) * 512], psb[:])
                    ypb_tiles.append(ypb)

                if debug_stop == "pre":
                    nc.sync.dma_start(dbg_xpt[:], xpT2[:])
                    for b in range(BL):
                        nc.sync.dma_start(dbg_ypb[b], ypb_tiles[b][:])

                if debug_stop is None:
                    # ---- outer products, 1MB fp16 tiles, stream out ----
                    out_pool = octx.enter_context(tc.tile_pool(name="outp", bufs=12))
                    for b in range(BL):
                        for c2 in range(C2):
                            ot = out_pool.tile([P, 2, D_OUT], f16, name="ot")
                            for jj in range(2):
                                nc.vector.tensor_scalar_mul(
                                    ot[:, jj, :],
                                    ypb_tiles[b][:],
                                    xpT2[:, c2, jj, b : b + 1],
                                )
                            nc.sync.dma_start(out_r[b, c2], ot[:])
            octx.close()

    nc.compile()
    return nc


def _get_nc():
    if "nc" not in _cache:
        _cache["nc"] = _build_nc()
    return _cache["nc"]


def _mask16():
    m = np.zeros((2 * BL, BL, P), dtype=np.float16)
    for b in range(BL):
        m[BL + b, b, :] = 1.0
    return m


def _in_maps(x, y, W):
    x = np.asarray(x, dtype=np.float32)
    y = np.asarray(y, dtype=np.float32)
    wt16 = np.ascontiguousarray(np.asarray(W, dtype=np.float32).T, dtype=np.float16)
    eye8 = np.eye(2 * BL, dtype=np.float32)
    mask = _mask16()
    maps = []
    for c in range(NCORES):
        xy = np.concatenate(
            [x[c * BL : (c + 1) * BL], y[c * BL : (c + 1) * BL]], axis=0
        )  # [8, 1024]
        # xyT16[p, k, b] = xy[b, k*128 + p]
        xyt = np.ascontiguousarray(
            xy.T.reshape(KC, P, 2 * BL).transpose(1, 0, 2), dtype=np.float16
        )
        maps.append({"xyt16": xyt, "wt": wt16, "eye8": eye8, "mask16": mask})
    return maps


def kernel(x: np.ndarray, y: np.ndarray, W: np.ndarray) -> np.ndarray:
    from concourse.bass_utils import run_bass_kernel_spmd

    nc = _get_nc()
    res = run_bass_kernel_spmd(nc, _in_maps(x, y, W), list(range(NCORES))).results
    return np.concatenate(
        [res[c]["out"].astype(np.float32) for c in range(NCORES)], axis=0
    )
